# revision 26
# baseline (speedup 1.0000x reference)
"""DMN encoder (3-hop masked-attention message passing) on 8 trn2 cores.

Sharding: pure data-parallel over the batch dim (16 rows/core).

v6 design (on top of v5's fp8 + host-transposed layouts):
  - the two row-groups' softmax chains are ZIPPED at op granularity, so
    each PE<->DVE round trip serves both groups and one group's DVE work
    hides under the other's stall.
  - per hop, the denominator sum and the vu-weighted sum ride ONE fused
    DVE reduce + ONE PE column-sum ([1, 2G]); the fp8 rescale row and the
    next hop's exp(c) row ride ONE PE broadcast matmul ([128, 2G]).
  - DMA: vt in 4 half-group slices then vn in 2 group slices on the sync
    FIFO ring (after the tiny params), pacing vsvu/passA starts.
"""
import sys

sys.path.insert(0, "/opt/trn_rl_repo")

import numpy as np
import ml_dtypes
import concourse.bass as bass
import concourse.tile as tile
from concourse import mybir
from concourse.bass_utils import run_bass_kernel_spmd
from contextlib import ExitStack

N_CORES = 8
B, N, D = 128, 2048, 128
BC = B // N_CORES          # batch rows per core
CH = N // 128              # neighbor chunks of 128
GB = 8                     # batch rows per pipeline group
NG = BC // GB
AF = mybir.ActivationFunctionType
ALU = mybir.AluOpType
FP32 = mybir.dt.float32
BF16 = mybir.dt.bfloat16
FP8 = mybir.dt.float8e4
CLAMP = 60.0               # overflow guard on exp() arguments
WS = 16.0                  # wfu pre-scale before fp8 quantization
P8 = 128.0                 # softmax-numerator fp8 scale

_mwctr = [0]


def _split_multiwaits(nc):
    """This walrus build rejects >1 sync-wait per instruction; hoist extras
    onto standalone EventSemaphore instructions on the same engine."""
    for fn in nc.m.functions:
        for bb in fn.blocks:
            new_list = []
            changed = False
            for ins in bb.instructions:
                si = getattr(ins, "sync_info", None)
                on_wait = list(si.on_wait) if si is not None else []
                if len(on_wait) > 1:
                    changed = True
                    for w in on_wait[:-1]:
                        _mwctr[0] += 1
                        ev = mybir.InstEventSemaphore(
                            name=f"I-mwfix-{_mwctr[0]}", ins=[], outs=[])
                        ev.engine = ins.engine
                        ev.debug = ins.debug
                        ev.sync_info = mybir.SyncInfo(on_wait=[w], on_update=[])
                        new_list.append(ev)
                        nc.register_instruction(ev, overwrite=True)
                    si.on_wait = [on_wait[-1]]
                    ins.sync_info = si
                new_list.append(ins)
            if changed:
                live = bb.instructions
                live[:] = new_list


def _build():
    nc = bass.Bass()
    vn_in = nc.dram_tensor("vn", [128, BC, CH, D], FP8, kind="ExternalInput")
    vt_in = nc.dram_tensor("vt", [128, BC, CH, 128], FP8,
                           kind="ExternalInput")
    mask_in = nc.dram_tensor("mask_t", [128, CH, BC], BF16,
                             kind="ExternalInput")
    e1_t = nc.dram_tensor("e1_t", [D, BC], FP32, kind="ExternalInput")
    w_lhsT = nc.dram_tensor("w_lhsT", [D, D], FP32, kind="ExternalInput")
    b_col = nc.dram_tensor("b_col", [D, 1], FP32, kind="ExternalInput")
    wfu_in = nc.dram_tensor("wfu", [D, 2], FP32, kind="ExternalInput")
    wfu8_in = nc.dram_tensor("wfu8", [D, 2], FP8, kind="ExternalInput")
    attb_in = nc.dram_tensor("attb", [1, 1], FP32, kind="ExternalInput")
    y = nc.dram_tensor("y", [D, BC], FP32, kind="ExternalOutput")

    with tile.TileContext(nc) as tc, ExitStack() as ctx:
        P = lambda **kw: ctx.enter_context(tc.tile_pool(**kw))
        sb = P(name="sb", bufs=1)                       # persistent singles
        wk = P(name="wk", bufs=3)                       # temporaries
        ps_vv = P(name="ps_vv", bufs=2, space="PSUM")   # vs/vu collectors
        ps_oA = P(name="ps_oA", bufs=2, space="PSUM")   # passA accumulators
        ps_oB = P(name="ps_oB", bufs=2, space="PSUM")   # passB accumulators
        ps_sm = P(name="ps_sm", bufs=2, space="PSUM")   # small matmul outs

        # ---- tiny params first: the sync HWDGE ring is FIFO, so these
        #      must precede the bulk V streams or compute waits on them ----
        wfu_sb = sb.tile([D, 2], FP32, tag="wfu")
        nc.sync.dma_start(out=wfu_sb, in_=wfu_in[:, :])
        wfu8_sb = sb.tile([D, 2], FP8, tag="wfu8")
        nc.sync.dma_start(out=wfu8_sb, in_=wfu8_in[:, :])
        attb_sb = sb.tile([1, 1], FP32, tag="attb")
        nc.sync.dma_start(out=attb_sb, in_=attb_in[:, :])
        u0 = sb.tile([D, BC], FP32, tag="u0")
        nc.sync.dma_start(out=u0, in_=e1_t[:, :])
        mask_sb = sb.tile([128, CH, BC], BF16, tag="mask")
        nc.sync.dma_start(out=mask_sb, in_=mask_in[:, :, :])
        w_sb = sb.tile([D, D], FP32, tag="w_sb")
        nc.scalar.dma_start(out=w_sb, in_=w_lhsT[:, :])
        bcol_sb = sb.tile([D, 1], FP32, tag="bcol")
        nc.scalar.dma_start(out=bcol_sb, in_=b_col[:, :])

        # ---- bulk V in need order: vt halves pace vsvu, vn groups
        #      land just before the passes need them ----
        vt_sb = sb.tile([128, BC, CH, 128], FP8, tag="vt")
        vn_sb = sb.tile([128, BC, CH, D], FP8, tag="vn")
        HG = GB // 2
        for q in range(2 * NG):
            hsl = slice(q * HG, (q + 1) * HG)
            nc.sync.dma_start(out=vt_sb[:, hsl, :, :], in_=vt_in[:, hsl, :, :])
        for g in range(NG):
            gsl = slice(g * GB, (g + 1) * GB)
            nc.sync.dma_start(out=vn_sb[:, gsl, :, :], in_=vn_in[:, gsl, :, :])

        vsvu = sb.tile([128, CH, BC, 2], FP32, tag="vsvu")
        E = sb.tile([128, CH, BC], BF16, tag="E")
        num01 = sb.tile([128, CH, BC, 2], FP8, tag="num01")
        num2 = sb.tile([128, CH, BC, 1], FP8, tag="num2")
        o01 = sb.tile([128, BC, 2], FP32, tag="o01")
        o2 = sb.tile([128, BC], FP32, tag="o2")
        ones_col = sb.tile([128, 1], BF16, tag="onesc")
        nc.vector.memset(ones_col, 1.0)
        ones_row = sb.tile([1, 128], FP32, tag="onesr")
        nc.vector.memset(ones_row, 1.0)

        # ---- helpers ----
        def bc_ap(row_ap):
            """[*, G] -> broadcast over the CH axis for [128, CH, G] ops."""
            return bass.AP(tensor=row_ap.tensor, offset=row_ap.offset,
                           ap=[row_ap.ap[0], [0, CH], row_ap.ap[1]])

        def dot_wu(rhs_tile):
            ps = ps_sm.tile([1, GB], FP32, tag="sm")
            nc.tensor.matmul(ps, lhsT=wfu_sb[:, 1:2], rhs=rhs_tile,
                             start=True, stop=True)
            return ps

        def lin_relu(u_tile, tg):
            ps = ps_sm.tile([D, GB], FP32, tag="sm")
            nc.tensor.matmul(ps, lhsT=w_sb, rhs=u_tile, start=True, stop=True)
            ub = wk.tile([D, GB], FP32, tag=f"ub{tg}")
            nc.scalar.activation(out=ub, in_=ps, func=AF.Relu,
                                 bias=bcol_sb, scale=1.0)
            return ub

        # ---- phase functions ----
        def prechain(g0, gn, gi):
            """u0-only work, hoisted ahead of the bulk phases. PSUM tiles
            from the shared pool must not stay live across phases (buffer
            rotation would deadlock the PE queue), so results are parked
            in SBUF immediately."""
            gsl = slice(g0, g0 + gn)
            u0g = u0[:, gsl]
            d0 = dot_wu(u0g)
            c0 = wk.tile([1, GB], FP32, tag=f"c0_{gi}")
            nc.vector.tensor_scalar(out=c0, in0=d0, scalar1=attb_sb,
                                    scalar2=None, op0=ALU.add)
            nc.vector.tensor_scalar_min(c0, c0, CLAMP)
            texp0 = wk.tile([1, GB], FP32, tag=f"texp0_{gi}")
            nc.scalar.activation(out=texp0, in_=c0, func=AF.Exp)
            t0ps = ps_sm.tile([128, GB], FP32, tag="sm")
            nc.tensor.matmul(t0ps, lhsT=ones_row, rhs=texp0,
                             start=True, stop=True)
            t0sb = wk.tile([128, GB], FP32, tag=f"t0sb_{gi}")
            nc.vector.tensor_copy(t0sb, t0ps)
            ub0 = lin_relu(u0g, f"0_{gi}")
            d1 = dot_wu(ub0)
            c1pre = wk.tile([1, GB], FP32, tag=f"c1p_{gi}")
            nc.vector.tensor_scalar(out=c1pre, in0=d1, scalar1=attb_sb,
                                    scalar2=None, op0=ALU.add)
            return dict(t0bc=t0sb, ub0=ub0, c1pre=c1pre, gsl=gsl, gi=gi,
                        g0=g0, gn=gn)

        def vsvu_phase(g0, gn, act_only):
            """act_only=True keeps all PSUM->SBUF copies off the DVE queue
            so the earlier group's softmax chain can start on DVE."""
            for b in range(g0, g0 + gn):
                acc = ps_vv.tile([128, 2 * CH], FP32, tag="accv")
                for c in range(CH):
                    nc.tensor.matmul(
                        acc[:, c * 2:(c + 1) * 2],
                        lhsT=vt_sb[:, b, c, :], rhs=wfu8_sb,
                        start=True, stop=True)
                if not act_only and b % 2 == 0:
                    nc.vector.tensor_scalar_mul(
                        vsvu[:, :, b, :],
                        acc.rearrange("p (c h) -> p c h", h=2), 1.0 / WS)
                else:
                    nc.scalar.activation(
                        out=vsvu[:, :, b, :],
                        in_=acc.rearrange("p (c h) -> p c h", h=2),
                        func=AF.Copy, scale=1.0 / WS)

        def hop_pair(sts, h):
            """One attention hop for both groups, ops zipped. h in {0, 1}.
            Needs st['t{h}bc'] (broadcast exp(c_h)); produces fp8-scaled
            num01[..., h]; h=0 also produces c1->t1bc, h=1 parks o1wu."""
            for st in sts:
                gsl, gi = st["gsl"], st["gi"]
                if h == 0:
                    # E holds exp(vs) * mask; the relu floor then becomes
                    # num = max(E*t, mask) - one fewer DVE op per hop
                    Eg = E[:, :, gsl]
                    nc.scalar.activation(out=Eg, in_=vsvu[:, :, gsl, 0],
                                         func=AF.Exp)
                    st["Eg"] = Eg
                    st["maskg"] = mask_sb[:, :, gsl]
                    nc.vector.tensor_tensor(out=Eg, in0=Eg, in1=st["maskg"],
                                            op=ALU.mult)
                    st["vu_g"] = vsvu[:, :, gsl, 1]
            for st in sts:
                gi = st["gi"]
                nb2 = wk.tile([128, CH, 2, GB], BF16, tag=f"nb2_{h}_{gi}")
                tmp = wk.tile([128, CH, GB], BF16, tag=f"numt{h}_{gi}")
                nc.vector.tensor_tensor(out=tmp, in0=st["Eg"],
                                        in1=bc_ap(st[f"t{h}bc"]),
                                        op=ALU.mult)
                nc.vector.tensor_tensor(out=nb2[:, :, 0, :], in0=tmp,
                                        in1=st["maskg"], op=ALU.max)
                nc.vector.tensor_tensor(out=nb2[:, :, 1, :],
                                        in0=nb2[:, :, 0, :],
                                        in1=st["vu_g"], op=ALU.mult)
                st["nb2"] = nb2
            for st in sts:
                ps = ps_sm.tile([1, 2 * GB], FP32, tag="sm")
                for c in range(CH):
                    nc.tensor.matmul(
                        ps, lhsT=ones_col,
                        rhs=st["nb2"][:, c, :, :].rearrange("p t b -> p (t b)"),
                        start=(c == 0), stop=(c == CH - 1))
                st["sums"] = ps
            for st in sts:
                gi = st["gi"]
                recip = wk.tile([1, GB], FP32, tag=f"recip{h}_{gi}")
                nc.vector.reciprocal(recip, st["sums"][:, 0:GB])
                owu = wk.tile([1, GB], FP32, tag=f"owu{h}_{gi}")
                nc.vector.tensor_tensor(out=owu, in0=st["sums"][:, GB:2 * GB],
                                        in1=recip, op=ALU.mult)
                # joint row: [recip*P8 | exp(c_next)] broadcast in one matmul
                jn = 2 * GB if h == 0 else GB
                joint = wk.tile([1, 2 * GB], FP32, tag=f"joint{h}_{gi}")
                nc.vector.tensor_scalar_mul(joint[:, 0:GB], recip, P8)
                if h == 0:
                    c1 = wk.tile([1, GB], FP32, tag=f"c1_{gi}")
                    nc.vector.tensor_tensor(out=c1, in0=st["c1pre"],
                                            in1=owu, op=ALU.add)
                    nc.vector.tensor_scalar_min(c1, c1, CLAMP)
                    nc.scalar.activation(out=joint[:, GB:2 * GB], in_=c1,
                                         func=AF.Exp)
                else:
                    st["o1wu"] = owu
                st["joint"] = joint[:, 0:jn]
            for st in sts:
                jn = st["joint"].shape[1]
                psb = ps_sm.tile([128, 2 * GB], FP32, tag="sm")
                nc.tensor.matmul(psb[:, 0:jn], lhsT=ones_row,
                                 rhs=st["joint"], start=True, stop=True)
                st["rsb"] = psb[:, 0:GB]
                if h == 0:
                    st["t1bc"] = psb[:, GB:2 * GB]
            for st in sts:
                gsl = st["gsl"]
                nc.vector.tensor_tensor(out=num01[:, :, gsl, h],
                                        in0=st["nb2"][:, :, 0, :],
                                        in1=bc_ap(st["rsb"]), op=ALU.mult)

        def passA(g0, gn):
            acc = ps_oA.tile([128, 2 * GB], FP32, tag="acca")
            for b in range(g0, g0 + gn):
                bb = b - g0
                for c in range(CH):
                    nc.tensor.matmul(
                        acc[:, bb * 2:(bb + 1) * 2],
                        lhsT=vn_sb[:, b, c, :], rhs=num01[:, c, b, :],
                        start=(c == 0), stop=(c == CH - 1))
            nc.vector.tensor_scalar_mul(
                o01[:, g0:g0 + gn, :],
                acc.rearrange("p (b h) -> p b h", h=2), 1.0 / P8)

        def chain2_pair(sts):
            for st in sts:
                gsl, gi = st["gsl"], st["gi"]
                u1 = wk.tile([D, GB], FP32, tag=f"u1_{gi}")
                nc.vector.tensor_tensor(out=u1, in0=o01[:, gsl, 0],
                                        in1=st["ub0"], op=ALU.add)
                st["u1"] = u1
            for st in sts:
                st["ub1"] = lin_relu(st["u1"], f"1_{st['gi']}")
            for st in sts:
                st["d2"] = dot_wu(st["ub1"])
            for st in sts:
                gsl, gi = st["gsl"], st["gi"]
                u2 = wk.tile([D, GB], FP32, tag=f"u2_{gi}")
                nc.vector.tensor_tensor(out=u2, in0=o01[:, gsl, 1],
                                        in1=st["ub1"], op=ALU.add)
                st["u2"] = u2
            for st in sts:
                gi = st["gi"]
                c2 = wk.tile([1, GB], FP32, tag=f"c2_{gi}")
                nc.vector.tensor_scalar(out=c2, in0=st["d2"],
                                        scalar1=attb_sb, scalar2=None,
                                        op0=ALU.add)
                nc.vector.tensor_tensor(out=c2, in0=c2, in1=st["o1wu"],
                                        op=ALU.add)
                nc.vector.tensor_scalar_min(c2, c2, CLAMP)
                st["c2"] = c2
            for st in sts:
                texp2 = wk.tile([1, GB], FP32, tag=f"texp2_{st['gi']}")
                nc.scalar.activation(out=texp2, in_=st["c2"], func=AF.Exp)
                st["texp2"] = texp2
            for st in sts:
                psb = ps_sm.tile([128, 2 * GB], FP32, tag="sm")
                nc.tensor.matmul(psb[:, 0:GB], lhsT=ones_row,
                                 rhs=st["texp2"], start=True, stop=True)
                st["t2bc"] = psb[:, 0:GB]
            for st in sts:
                gi = st["gi"]
                nb2 = wk.tile([128, CH, GB], BF16, tag=f"nb2_2_{gi}")
                nc.vector.tensor_tensor(out=nb2, in0=st["Eg"],
                                        in1=bc_ap(st["t2bc"]), op=ALU.mult)
                nc.vector.tensor_tensor(out=nb2, in0=nb2,
                                        in1=st["maskg"], op=ALU.max)
                st["nb2c2"] = nb2
            for st in sts:
                ps = ps_sm.tile([1, 2 * GB], FP32, tag="sm")
                for c in range(CH):
                    nc.tensor.matmul(ps[:, 0:GB], lhsT=ones_col,
                                     rhs=st["nb2c2"][:, c, :],
                                     start=(c == 0), stop=(c == CH - 1))
                st["sum2"] = ps
            for st in sts:
                gi = st["gi"]
                rs = wk.tile([1, GB], FP32, tag=f"rs2_{gi}")
                nc.vector.reciprocal(rs, st["sum2"][:, 0:GB])
                nc.vector.tensor_scalar_mul(rs, rs, P8)
                st["rs2"] = rs
            for st in sts:
                psb = ps_sm.tile([128, 2 * GB], FP32, tag="sm")
                nc.tensor.matmul(psb[:, 0:GB], lhsT=ones_row, rhs=st["rs2"],
                                 start=True, stop=True)
                st["rsb2"] = psb[:, 0:GB]
            for st in sts:
                gsl = st["gsl"]
                nc.vector.tensor_tensor(out=num2[:, :, gsl, 0],
                                        in0=st["nb2c2"],
                                        in1=bc_ap(st["rsb2"]), op=ALU.mult)
            for st in sts:
                st["ub2"] = lin_relu(st["u2"], f"2_{st['gi']}")

        def passB(g0, gn):
            acc = ps_oB.tile([128, GB], FP32, tag="accb")
            for b in range(g0, g0 + gn):
                bb = b - g0
                for c in range(CH):
                    nc.tensor.matmul(
                        acc[:, bb:bb + 1],
                        lhsT=vn_sb[:, b, c, :], rhs=num2[:, c, b, :],
                        start=(c == 0), stop=(c == CH - 1))
            nc.scalar.activation(out=o2[:, g0:g0 + gn], in_=acc,
                                 func=AF.Copy, scale=1.0 / P8)

        def finish(st, g0, gn):
            gsl = st["gsl"]
            u3 = wk.tile([D, GB], FP32, tag=f"u3_{st['gi']}")
            nc.vector.tensor_tensor(out=u3, in0=o2[:, gsl],
                                    in1=st["ub2"], op=ALU.add)
            nc.sync.dma_start(out=y[:, gsl], in_=u3)

        # ---- pipeline ----
        sts = [prechain(g * GB, GB, g) for g in range(NG)]
        for g in range(NG):
            vsvu_phase(g * GB, GB, act_only=(g > 0))
        hop_pair(sts, 0)
        hop_pair(sts, 1)
        # group 0's pass A -> hop-2 chain -> pass B run while group 1's
        # pass A is still waiting on its vn DMA slice
        passA(0, GB)
        chain2_pair(sts[0:1])
        passB(0, GB)
        passA(GB, GB)
        chain2_pair(sts[1:2])
        passB(GB, GB)
        for g in range(NG):
            finish(sts[g], g * GB, GB)

    _split_multiwaits(nc)
    return nc


_nc_cache = None


def _get_nc():
    global _nc_cache
    if _nc_cache is None:
        _nc_cache = _build()
    return _nc_cache


def make_in_maps(inputs):
    e1 = np.asarray(inputs["e1_embeded"], dtype=np.float32)
    value = np.asarray(inputs["nei_embeded_value"], dtype=np.float32)
    mask = np.asarray(inputs["nei_mask"], dtype=np.float32)
    linfc_w = np.asarray(inputs["linfc_w"], dtype=np.float32)
    linfc_b = np.asarray(inputs["linfc_b"], dtype=np.float32)
    attfc_w = np.asarray(inputs["attfc_w"], dtype=np.float32)
    attfc_b = np.asarray(inputs["attfc_b"], dtype=np.float32)

    bf16 = ml_dtypes.bfloat16
    f8 = ml_dtypes.float8_e4m3
    w_lhsT = np.ascontiguousarray(linfc_w.T)
    b_col = np.ascontiguousarray(linfc_b.reshape(D, 1))
    wfu = np.ascontiguousarray(
        np.stack([attfc_w[0, :D], attfc_w[0, D:]], axis=1))
    wfu8 = (wfu * WS).astype(f8)
    attb = np.asarray(attfc_b, dtype=np.float32).reshape(1, 1)

    in_maps = []
    for core in range(N_CORES):
        b0 = core * BC
        r = value[b0:b0 + BC].reshape(BC, 128, CH, D)
        in_maps.append({
            "vn": r.transpose(1, 0, 2, 3).astype(f8),
            "vt": r.transpose(3, 0, 2, 1).astype(f8),
            "mask_t": mask[b0:b0 + BC].reshape(BC, 128, CH)
                      .transpose(1, 2, 0).astype(bf16),
            "e1_t": np.ascontiguousarray(e1[b0:b0 + BC].T),
            "w_lhsT": w_lhsT,
            "b_col": b_col,
            "wfu": wfu,
            "wfu8": wfu8,
            "attb": attb,
        })
    return in_maps


def kernel(**inputs):
    in_maps = make_in_maps(inputs)
    nc = _get_nc()
    res = run_bass_kernel_spmd(nc, in_maps, list(range(N_CORES)))
    out = np.concatenate(
        [np.asarray(res.results[i]["y"]).T for i in range(N_CORES)], axis=0)
    return np.ascontiguousarray(out, dtype=np.float32)


# revision 28
# speedup vs baseline: 1.0614x; 1.0614x over previous
"""DMN encoder (3-hop masked-attention message passing) on 8 trn2 cores.

Sharding: pure data-parallel over the batch dim (16 rows/core). 130us
baseline -> ~66.6us measured.

Design:
  - host pre-casts V to fp8e4m3 in BOTH layouts (vn: neighbors-on-
    partitions for the o-passes, vt: d-on-partitions for the vs/vu dots):
    8.4 MB HBM per core via plain HWDGE DMA - no SWDGE casts and no PE
    transposes. wfu is pre-scaled by 16 before fp8 (entries ~N(0,1/256)
    would hit subnormals); the 1/16 descale rides the PSUM->SBUF copy.
  - DMA order on the sync FIFO ring: tiny params first (the ring is
    FIFO - params queued behind bulk stall all compute), then vt in
    half-group slices pacing vsvu, then vn per group for the passes.
  - vs/vu dots: lhsT = vt chunk (stationary, fp8 FWL), rhs = wfu8
    -> out [128(p), 2] lands directly in softmax layout.
  - o-passes: lhsT = vn chunk, rhs = fp8 num columns -> out [128(d), h]
    accumulated across chunks in PSUM; numerators are pre-scaled by
    recip*128 so the descaled PSUM result IS o, and u-updates are adds.
  - denominator epsilon (1e-5) dropped: the masked-softmax numerator
    always contains its own max so denom >= O(0.1); error ~1e-4 rel.
    This kills the masked-max partition-reduce chain entirely.
  - E holds exp(vs)*mask so the relu floor is num = max(E*t, mask).
  - per hop, denominator + vu-weighted sums ride chunk-accumulated PE
    matmuls (ones lhsT) instead of a DVE reduce; the fp8 rescale row and
    next hop's exp(c) row share ONE PE broadcast matmul.
  - the two row-groups' chains are zipped at op granularity so each
    PE<->DVE round trip serves both groups; u-only matmuls (u.wu, linfc)
    are hoisted ahead of the bulk phases.
  - y is written [D, BC] (no output transpose); host transposes.
"""
import sys

sys.path.insert(0, "/opt/trn_rl_repo")

import numpy as np
import ml_dtypes
import concourse.bass as bass
import concourse.tile as tile
from concourse import mybir
from concourse.bass_utils import run_bass_kernel_spmd
from contextlib import ExitStack

N_CORES = 8
B, N, D = 128, 2048, 128
BC = B // N_CORES          # batch rows per core
CH = N // 128              # neighbor chunks of 128
GB = 8                     # batch rows per pipeline group
NG = BC // GB
AF = mybir.ActivationFunctionType
ALU = mybir.AluOpType
FP32 = mybir.dt.float32
BF16 = mybir.dt.bfloat16
FP8 = mybir.dt.float8e4
CLAMP = 60.0               # overflow guard on exp() arguments
WS = 16.0                  # wfu pre-scale before fp8 quantization
P8 = 128.0                 # softmax-numerator fp8 scale

_mwctr = [0]


def _split_multiwaits(nc):
    """This walrus build rejects >1 sync-wait per instruction; hoist extras
    onto standalone EventSemaphore instructions on the same engine."""
    for fn in nc.m.functions:
        for bb in fn.blocks:
            new_list = []
            changed = False
            for ins in bb.instructions:
                si = getattr(ins, "sync_info", None)
                on_wait = list(si.on_wait) if si is not None else []
                if len(on_wait) > 1:
                    changed = True
                    for w in on_wait[:-1]:
                        _mwctr[0] += 1
                        ev = mybir.InstEventSemaphore(
                            name=f"I-mwfix-{_mwctr[0]}", ins=[], outs=[])
                        ev.engine = ins.engine
                        ev.debug = ins.debug
                        ev.sync_info = mybir.SyncInfo(on_wait=[w], on_update=[])
                        new_list.append(ev)
                        nc.register_instruction(ev, overwrite=True)
                    si.on_wait = [on_wait[-1]]
                    ins.sync_info = si
                new_list.append(ins)
            if changed:
                live = bb.instructions
                live[:] = new_list


def _build():
    nc = bass.Bass()
    vn_in = nc.dram_tensor("vn", [128, BC, CH, D], FP8, kind="ExternalInput")
    vt_in = nc.dram_tensor("vt", [128, BC, CH, 128], FP8,
                           kind="ExternalInput")
    mask_in = nc.dram_tensor("mask_t", [128, CH, BC], BF16,
                             kind="ExternalInput")
    e1_t = nc.dram_tensor("e1_t", [D, BC], FP32, kind="ExternalInput")
    w_lhsT = nc.dram_tensor("w_lhsT", [D, D], FP32, kind="ExternalInput")
    b_col = nc.dram_tensor("b_col", [D, 1], FP32, kind="ExternalInput")
    wfu_in = nc.dram_tensor("wfu", [D, 2], FP32, kind="ExternalInput")
    wfu8_in = nc.dram_tensor("wfu8", [D, 2], FP8, kind="ExternalInput")
    attb_in = nc.dram_tensor("attb", [1, 1], FP32, kind="ExternalInput")
    y = nc.dram_tensor("y", [D, BC], FP32, kind="ExternalOutput")

    with tile.TileContext(nc) as tc, ExitStack() as ctx:
        P = lambda **kw: ctx.enter_context(tc.tile_pool(**kw))
        sb = P(name="sb", bufs=1)                       # persistent singles
        wk = P(name="wk", bufs=3)                       # temporaries
        ps_vv = P(name="ps_vv", bufs=2, space="PSUM")   # vs/vu collectors
        ps_oA = P(name="ps_oA", bufs=2, space="PSUM")   # passA accumulators
        ps_oB = P(name="ps_oB", bufs=2, space="PSUM")   # passB accumulators
        ps_sm = P(name="ps_sm", bufs=2, space="PSUM")   # small matmul outs

        # ---- tiny params first: the sync HWDGE ring is FIFO, so these
        #      must precede the bulk V streams or compute waits on them ----
        wfu_sb = sb.tile([D, 2], FP32, tag="wfu")
        nc.sync.dma_start(out=wfu_sb, in_=wfu_in[:, :])
        wfu8_sb = sb.tile([D, 2], FP8, tag="wfu8")
        nc.sync.dma_start(out=wfu8_sb, in_=wfu8_in[:, :])
        attb_sb = sb.tile([1, 1], FP32, tag="attb")
        nc.sync.dma_start(out=attb_sb, in_=attb_in[:, :])
        u0 = sb.tile([D, BC], FP32, tag="u0")
        nc.sync.dma_start(out=u0, in_=e1_t[:, :])
        mask_sb = sb.tile([128, CH, BC], BF16, tag="mask")
        nc.sync.dma_start(out=mask_sb, in_=mask_in[:, :, :])
        w_sb = sb.tile([D, D], FP32, tag="w_sb")
        nc.scalar.dma_start(out=w_sb, in_=w_lhsT[:, :])
        bcol_sb = sb.tile([D, 1], FP32, tag="bcol")
        nc.scalar.dma_start(out=bcol_sb, in_=b_col[:, :])

        # ---- bulk V in need order: vt halves pace vsvu, vn groups
        #      land just before the passes need them ----
        vt_sb = sb.tile([128, BC, CH, 128], FP8, tag="vt")
        vn_sb = sb.tile([128, BC, CH, D], FP8, tag="vn")
        HG = GB // 2
        for q in range(2 * NG):
            hsl = slice(q * HG, (q + 1) * HG)
            nc.sync.dma_start(out=vt_sb[:, hsl, :, :], in_=vt_in[:, hsl, :, :])
        for g in range(NG):
            gsl = slice(g * GB, (g + 1) * GB)
            nc.sync.dma_start(out=vn_sb[:, gsl, :, :], in_=vn_in[:, gsl, :, :])

        vsvu = sb.tile([128, CH, BC, 2], FP32, tag="vsvu")
        E = sb.tile([128, CH, BC], BF16, tag="E")
        num01 = sb.tile([128, CH, BC, 2], FP8, tag="num01")
        num2 = sb.tile([128, CH, BC, 1], FP8, tag="num2")
        o01 = sb.tile([128, BC, 2], FP32, tag="o01")
        o2 = sb.tile([128, BC], FP32, tag="o2")
        ones_col = sb.tile([128, 1], BF16, tag="onesc")
        nc.vector.memset(ones_col, 1.0)
        ones_row = sb.tile([1, 128], FP32, tag="onesr")
        nc.vector.memset(ones_row, 1.0)

        # ---- helpers ----
        def bc_ap(row_ap):
            """[*, G] -> broadcast over the CH axis for [128, CH, G] ops."""
            return bass.AP(tensor=row_ap.tensor, offset=row_ap.offset,
                           ap=[row_ap.ap[0], [0, CH], row_ap.ap[1]])

        def dot_wu(rhs_tile):
            ps = ps_sm.tile([1, GB], FP32, tag="sm")
            nc.tensor.matmul(ps, lhsT=wfu_sb[:, 1:2], rhs=rhs_tile,
                             start=True, stop=True)
            return ps

        def lin_relu(u_tile, tg):
            ps = ps_sm.tile([D, GB], FP32, tag="sm")
            nc.tensor.matmul(ps, lhsT=w_sb, rhs=u_tile, start=True, stop=True)
            ub = wk.tile([D, GB], FP32, tag=f"ub{tg}")
            nc.scalar.activation(out=ub, in_=ps, func=AF.Relu,
                                 bias=bcol_sb, scale=1.0)
            return ub

        # ---- phase functions ----
        def prechain(g0, gn, gi):
            """u0-only work, hoisted ahead of the bulk phases. PSUM tiles
            from the shared pool must not stay live across phases (buffer
            rotation would deadlock the PE queue), so results are parked
            in SBUF immediately."""
            gsl = slice(g0, g0 + gn)
            u0g = u0[:, gsl]
            d0 = dot_wu(u0g)
            c0 = wk.tile([1, GB], FP32, tag=f"c0_{gi}")
            nc.vector.tensor_scalar(out=c0, in0=d0, scalar1=attb_sb,
                                    scalar2=None, op0=ALU.add)
            nc.vector.tensor_scalar_min(c0, c0, CLAMP)
            texp0 = wk.tile([1, GB], FP32, tag=f"texp0_{gi}")
            nc.scalar.activation(out=texp0, in_=c0, func=AF.Exp)
            t0ps = ps_sm.tile([128, GB], FP32, tag="sm")
            nc.tensor.matmul(t0ps, lhsT=ones_row, rhs=texp0,
                             start=True, stop=True)
            t0sb = wk.tile([128, GB], FP32, tag=f"t0sb_{gi}")
            nc.vector.tensor_copy(t0sb, t0ps)
            ub0 = lin_relu(u0g, f"0_{gi}")
            d1 = dot_wu(ub0)
            c1pre = wk.tile([1, GB], FP32, tag=f"c1p_{gi}")
            nc.vector.tensor_scalar(out=c1pre, in0=d1, scalar1=attb_sb,
                                    scalar2=None, op0=ALU.add)
            return dict(t0bc=t0sb, ub0=ub0, c1pre=c1pre, gsl=gsl, gi=gi,
                        g0=g0, gn=gn)

        def vsvu_phase(g0, gn, act_only):
            """act_only=True keeps all PSUM->SBUF copies off the DVE queue
            so the earlier group's softmax chain can start on DVE."""
            for b in range(g0, g0 + gn):
                acc = ps_vv.tile([128, 2 * CH], FP32, tag="accv")
                for c in range(CH):
                    nc.tensor.matmul(
                        acc[:, c * 2:(c + 1) * 2],
                        lhsT=vt_sb[:, b, c, :], rhs=wfu8_sb,
                        start=True, stop=True)
                if not act_only and b % 2 == 0:
                    nc.vector.tensor_scalar_mul(
                        vsvu[:, :, b, :],
                        acc.rearrange("p (c h) -> p c h", h=2), 1.0 / WS)
                else:
                    nc.scalar.activation(
                        out=vsvu[:, :, b, :],
                        in_=acc.rearrange("p (c h) -> p c h", h=2),
                        func=AF.Copy, scale=1.0 / WS)

        def hop_pair(sts, h):
            """One attention hop for both groups, ops zipped. h in {0, 1}.
            Needs st['t{h}bc'] (broadcast exp(c_h)); produces fp8-scaled
            num01[..., h]; h=0 also produces c1->t1bc, h=1 parks o1wu."""
            for st in sts:
                gsl, gi = st["gsl"], st["gi"]
                if h == 0:
                    # E holds exp(vs) * mask; the relu floor then becomes
                    # num = max(E*t, mask) - one fewer DVE op per hop
                    Eg = E[:, :, gsl]
                    nc.scalar.activation(out=Eg, in_=vsvu[:, :, gsl, 0],
                                         func=AF.Exp)
                    st["Eg"] = Eg
                    st["maskg"] = mask_sb[:, :, gsl]
                    nc.vector.tensor_tensor(out=Eg, in0=Eg, in1=st["maskg"],
                                            op=ALU.mult)
                    st["vu_g"] = vsvu[:, :, gsl, 1]
            for st in sts:
                gi = st["gi"]
                nb2 = wk.tile([128, CH, 2, GB], BF16, tag=f"nb2_{h}_{gi}")
                tmp = wk.tile([128, CH, GB], BF16, tag=f"numt{h}_{gi}")
                nc.vector.tensor_tensor(out=tmp, in0=st["Eg"],
                                        in1=bc_ap(st[f"t{h}bc"]),
                                        op=ALU.mult)
                nc.vector.tensor_tensor(out=nb2[:, :, 0, :], in0=tmp,
                                        in1=st["maskg"], op=ALU.max)
                nc.vector.tensor_tensor(out=nb2[:, :, 1, :],
                                        in0=nb2[:, :, 0, :],
                                        in1=st["vu_g"], op=ALU.mult)
                st["nb2"] = nb2
            for st in sts:
                ps = ps_sm.tile([1, 2 * GB], FP32, tag="sm")
                for c in range(CH):
                    nc.tensor.matmul(
                        ps, lhsT=ones_col,
                        rhs=st["nb2"][:, c, :, :].rearrange("p t b -> p (t b)"),
                        start=(c == 0), stop=(c == CH - 1))
                st["sums"] = ps
            for st in sts:
                gi = st["gi"]
                recip = wk.tile([1, GB], FP32, tag=f"recip{h}_{gi}")
                nc.vector.reciprocal(recip, st["sums"][:, 0:GB])
                owu = wk.tile([1, GB], FP32, tag=f"owu{h}_{gi}")
                nc.vector.tensor_tensor(out=owu, in0=st["sums"][:, GB:2 * GB],
                                        in1=recip, op=ALU.mult)
                # joint row: [recip*P8 | exp(c_next)] broadcast in one matmul
                jn = 2 * GB if h == 0 else GB
                joint = wk.tile([1, 2 * GB], FP32, tag=f"joint{h}_{gi}")
                nc.vector.tensor_scalar_mul(joint[:, 0:GB], recip, P8)
                if h == 0:
                    c1 = wk.tile([1, GB], FP32, tag=f"c1_{gi}")
                    nc.vector.tensor_tensor(out=c1, in0=st["c1pre"],
                                            in1=owu, op=ALU.add)
                    nc.vector.tensor_scalar_min(c1, c1, CLAMP)
                    nc.scalar.activation(out=joint[:, GB:2 * GB], in_=c1,
                                         func=AF.Exp)
                else:
                    st["o1wu"] = owu
                st["joint"] = joint[:, 0:jn]
            for st in sts:
                jn = st["joint"].shape[1]
                psb = ps_sm.tile([128, 2 * GB], FP32, tag="sm")
                nc.tensor.matmul(psb[:, 0:jn], lhsT=ones_row,
                                 rhs=st["joint"], start=True, stop=True)
                st["rsb"] = psb[:, 0:GB]
                if h == 0:
                    st["t1bc"] = psb[:, GB:2 * GB]
            for st in sts:
                gsl = st["gsl"]
                nc.vector.tensor_tensor(out=num01[:, :, gsl, h],
                                        in0=st["nb2"][:, :, 0, :],
                                        in1=bc_ap(st["rsb"]), op=ALU.mult)

        def passA(g0, gn):
            acc = ps_oA.tile([128, 2 * GB], FP32, tag="acca")
            for b in range(g0, g0 + gn):
                bb = b - g0
                for c in range(CH):
                    nc.tensor.matmul(
                        acc[:, bb * 2:(bb + 1) * 2],
                        lhsT=vn_sb[:, b, c, :], rhs=num01[:, c, b, :],
                        start=(c == 0), stop=(c == CH - 1))
            nc.vector.tensor_scalar_mul(
                o01[:, g0:g0 + gn, :],
                acc.rearrange("p (b h) -> p b h", h=2), 1.0 / P8)

        def chain2_pair(sts):
            for st in sts:
                gsl, gi = st["gsl"], st["gi"]
                u1 = wk.tile([D, GB], FP32, tag=f"u1_{gi}")
                nc.vector.tensor_tensor(out=u1, in0=o01[:, gsl, 0],
                                        in1=st["ub0"], op=ALU.add)
                st["u1"] = u1
            for st in sts:
                st["ub1"] = lin_relu(st["u1"], f"1_{st['gi']}")
            for st in sts:
                st["d2"] = dot_wu(st["ub1"])
            for st in sts:
                gsl, gi = st["gsl"], st["gi"]
                u2 = wk.tile([D, GB], FP32, tag=f"u2_{gi}")
                nc.vector.tensor_tensor(out=u2, in0=o01[:, gsl, 1],
                                        in1=st["ub1"], op=ALU.add)
                st["u2"] = u2
            for st in sts:
                gi = st["gi"]
                c2 = wk.tile([1, GB], FP32, tag=f"c2_{gi}")
                nc.vector.tensor_scalar(out=c2, in0=st["d2"],
                                        scalar1=attb_sb, scalar2=None,
                                        op0=ALU.add)
                nc.vector.tensor_tensor(out=c2, in0=c2, in1=st["o1wu"],
                                        op=ALU.add)
                nc.vector.tensor_scalar_min(c2, c2, CLAMP)
                st["c2"] = c2
            for st in sts:
                texp2 = wk.tile([1, GB], FP32, tag=f"texp2_{st['gi']}")
                nc.scalar.activation(out=texp2, in_=st["c2"], func=AF.Exp)
                st["texp2"] = texp2
            for st in sts:
                psb = ps_sm.tile([128, 2 * GB], FP32, tag="sm")
                nc.tensor.matmul(psb[:, 0:GB], lhsT=ones_row,
                                 rhs=st["texp2"], start=True, stop=True)
                st["t2bc"] = psb[:, 0:GB]
            for st in sts:
                gi = st["gi"]
                nb2 = wk.tile([128, CH, GB], BF16, tag=f"nb2_2_{gi}")
                nc.vector.tensor_tensor(out=nb2, in0=st["Eg"],
                                        in1=bc_ap(st["t2bc"]), op=ALU.mult)
                nc.vector.tensor_tensor(out=nb2, in0=nb2,
                                        in1=st["maskg"], op=ALU.max)
                st["nb2c2"] = nb2
            for st in sts:
                ps = ps_sm.tile([1, 2 * GB], FP32, tag="sm")
                for c in range(CH):
                    nc.tensor.matmul(ps[:, 0:GB], lhsT=ones_col,
                                     rhs=st["nb2c2"][:, c, :],
                                     start=(c == 0), stop=(c == CH - 1))
                st["sum2"] = ps
            for st in sts:
                gi = st["gi"]
                rs = wk.tile([1, GB], FP32, tag=f"rs2_{gi}")
                nc.vector.reciprocal(rs, st["sum2"][:, 0:GB])
                nc.vector.tensor_scalar_mul(rs, rs, P8)
                st["rs2"] = rs
            for st in sts:
                psb = ps_sm.tile([128, 2 * GB], FP32, tag="sm")
                nc.tensor.matmul(psb[:, 0:GB], lhsT=ones_row, rhs=st["rs2"],
                                 start=True, stop=True)
                st["rsb2"] = psb[:, 0:GB]
            for st in sts:
                gsl = st["gsl"]
                nc.vector.tensor_tensor(out=num2[:, :, gsl, 0],
                                        in0=st["nb2c2"],
                                        in1=bc_ap(st["rsb2"]), op=ALU.mult)
            for st in sts:
                st["ub2"] = lin_relu(st["u2"], f"2_{st['gi']}")

        def passB(g0, gn):
            acc = ps_oB.tile([128, GB], FP32, tag="accb")
            for b in range(g0, g0 + gn):
                bb = b - g0
                for c in range(CH):
                    nc.tensor.matmul(
                        acc[:, bb:bb + 1],
                        lhsT=vn_sb[:, b, c, :], rhs=num2[:, c, b, :],
                        start=(c == 0), stop=(c == CH - 1))
            nc.scalar.activation(out=o2[:, g0:g0 + gn], in_=acc,
                                 func=AF.Copy, scale=1.0 / P8)

        def finish(st, g0, gn):
            gsl = st["gsl"]
            u3 = wk.tile([D, GB], FP32, tag=f"u3_{st['gi']}")
            nc.vector.tensor_tensor(out=u3, in0=o2[:, gsl],
                                    in1=st["ub2"], op=ALU.add)
            nc.sync.dma_start(out=y[:, gsl], in_=u3)

        # ---- pipeline ----
        sts = [prechain(g * GB, GB, g) for g in range(NG)]
        for g in range(NG):
            vsvu_phase(g * GB, GB, act_only=(g > 0))
        hop_pair(sts, 0)
        hop_pair(sts, 1)
        for g in range(NG):
            passA(g * GB, GB)
        chain2_pair(sts)
        for g in range(NG):
            passB(g * GB, GB)
        for g in range(NG):
            finish(sts[g], g * GB, GB)

    _split_multiwaits(nc)
    return nc


_nc_cache = None


def _get_nc():
    global _nc_cache
    if _nc_cache is None:
        _nc_cache = _build()
    return _nc_cache


def make_in_maps(inputs):
    e1 = np.asarray(inputs["e1_embeded"], dtype=np.float32)
    value = np.asarray(inputs["nei_embeded_value"], dtype=np.float32)
    mask = np.asarray(inputs["nei_mask"], dtype=np.float32)
    linfc_w = np.asarray(inputs["linfc_w"], dtype=np.float32)
    linfc_b = np.asarray(inputs["linfc_b"], dtype=np.float32)
    attfc_w = np.asarray(inputs["attfc_w"], dtype=np.float32)
    attfc_b = np.asarray(inputs["attfc_b"], dtype=np.float32)

    bf16 = ml_dtypes.bfloat16
    f8 = ml_dtypes.float8_e4m3
    w_lhsT = np.ascontiguousarray(linfc_w.T)
    b_col = np.ascontiguousarray(linfc_b.reshape(D, 1))
    wfu = np.ascontiguousarray(
        np.stack([attfc_w[0, :D], attfc_w[0, D:]], axis=1))
    wfu8 = (wfu * WS).astype(f8)
    attb = np.asarray(attfc_b, dtype=np.float32).reshape(1, 1)

    in_maps = []
    for core in range(N_CORES):
        b0 = core * BC
        r = value[b0:b0 + BC].reshape(BC, 128, CH, D)
        in_maps.append({
            "vn": r.transpose(1, 0, 2, 3).astype(f8),
            "vt": r.transpose(3, 0, 2, 1).astype(f8),
            "mask_t": mask[b0:b0 + BC].reshape(BC, 128, CH)
                      .transpose(1, 2, 0).astype(bf16),
            "e1_t": np.ascontiguousarray(e1[b0:b0 + BC].T),
            "w_lhsT": w_lhsT,
            "b_col": b_col,
            "wfu": wfu,
            "wfu8": wfu8,
            "attb": attb,
        })
    return in_maps


def kernel(**inputs):
    in_maps = make_in_maps(inputs)
    nc = _get_nc()
    res = run_bass_kernel_spmd(nc, in_maps, list(range(N_CORES)))
    out = np.concatenate(
        [np.asarray(res.results[i]["y"]).T for i in range(N_CORES)], axis=0)
    return np.ascontiguousarray(out, dtype=np.float32)
